# revision 1
# baseline (speedup 1.0000x reference)
"""HDC binary attention kernel for 8 trn2 NeuronCores.

Problem: B,T,D = 4,2048,1024
    Q = sign(x * sign(bv_q)); K = sign(x * sign(bv_k)); V = x * sign(bv_v)
    scores = (Q @ K^T) / sqrt(D), causal
    out = sigmoid(4*scores) * causal_mask @ V

Math used by the kernel:
    sign(x*bq) = sign(x)*sign(bq) elementwise, so with S = sign(x) (+-1) and
    c[d] = sign(bv_q)[d]*sign(bv_k)[d]:
        scores[t,s] = sum_d S[t,d]*c[d]*S[s,d] / 32
    We compute scores TRANSPOSED (s on partitions) via
        scoresT = SkT.T @ (c * SqT)   (contraction d on partitions, bf16 exact)
    then attnT = sigmoid(scoresT * 0.125) (* mask on diagonal chunks), fp16,
    and out = attnT.T @ V accumulated over s-subtiles (fp16 matmul).

Sharding: 2 cores per batch. Each 512-row chunk of T is split in half:
    core parity 0 takes rows [512j, 512j+256), parity 1 takes [512j+256, 512j+512).
For SPMD uniformity the host permutes K/V rows for parity-1 cores (swapping the
halves of every 512-chunk) so that each core's q rows always sit at canonical
positions [512j, 512j+256); causal boundary handling is via host-built masks.
Each q group j attends to canonical s < 512*(j+1); full 512-chunks below the
boundary are permutation-invariant, the boundary chunk is masked explicitly.
"""

import numpy as np

B, T, D = 4, 2048, 1024
NQ = 1024          # q rows per core
NCORES = 8
ST = 16            # s-tiles of 128 rows
DT = 8             # d-tiles of 128
NG = 4             # q groups of 256 rows per core

_CACHE = {}


def build_nc():
    """Build + schedule + compile the (single, SPMD-uniform) bass program."""
    import concourse.bass as bass
    import concourse.bacc as bacc
    import concourse.mybir as mybir
    import concourse.tile as tile

    fp32 = mybir.dt.float32
    bf16 = mybir.dt.bfloat16
    fp16 = mybir.dt.float16
    AF = mybir.ActivationFunctionType

    nc = bacc.Bacc("TRN2", target_bir_lowering=False, debug=False)

    xk_d = nc.dram_tensor("xk", [T, D], fp32, kind="ExternalInput").ap()
    cmat_d = nc.dram_tensor("cmat", [128, DT], fp32, kind="ExternalInput").ap()
    bvs_d = nc.dram_tensor("bvs", [128, D], fp32, kind="ExternalInput").ap()
    # maskt[wq][p, ct]: keep for boundary s-offset (128*wq+p) vs q col offset ct
    mask_d = nc.dram_tensor("maskt", [4, 128, 256], fp16, kind="ExternalInput").ap()
    ident_d = nc.dram_tensor("ident", [128, 128], bf16, kind="ExternalInput").ap()
    out_d = nc.dram_tensor("out", [NQ, D], fp32, kind="ExternalOutput").ap()

    with tile.TileContext(nc) as tc:
        with (
            tc.tile_pool(name="const", bufs=1) as constp,
            tc.tile_pool(name="load", bufs=6) as loadp,
            tc.tile_pool(name="kt", bufs=1) as ktp,
            tc.tile_pool(name="qt", bufs=1) as qtp,
            tc.tile_pool(name="vv", bufs=1) as vvp,
            tc.tile_pool(name="at", bufs=1) as atp,
            tc.tile_pool(name="ps", bufs=3, space="PSUM") as psp,
            tc.tile_pool(name="po", bufs=2, space="PSUM") as pop,
            tc.tile_pool(name="pt", bufs=3, space="PSUM") as ptp,
            tc.tile_pool(name="outb", bufs=3) as outp,
        ):
            # ---- constants ----
            bvs_sb = constp.tile([128, D], fp32, tag="bvs")
            nc.gpsimd.dma_start(bvs_sb[:], bvs_d)
            cmat_sb = constp.tile([128, DT], fp32, tag="cmat")
            nc.gpsimd.dma_start(cmat_sb[:], cmat_d)
            mask_sb = [constp.tile([128, 256], fp16, tag=f"mask{w}", name=f"mask{w}") for w in range(4)]
            for w in range(4):
                nc.gpsimd.dma_start(mask_sb[w][:], mask_d[w])
            ident_sb = constp.tile([128, 128], bf16, tag="ident")
            nc.gpsimd.dma_start(ident_sb[:], ident_d)

            # ---- persistent per-s-tile arrays ----
            # SkT[st]: [128 d-part, 8*128] bf16; cols dk*128+j = S^T[d=128dk+p, s=128st+j]
            skt = [ktp.tile([128, DT * 128], bf16, tag=f"skt{st}", name=f"skt{st}") for st in range(ST)]
            # ScqT[g]: [128 d-part, 8*256] bf16; cols dk*256+ct = c*S^T at q col (256g+ct)
            scq = [qtp.tile([128, DT * 256], bf16, tag=f"scq{g}", name=f"scq{g}") for g in range(NG)]
            # V[st]: [128 s-part, 1024 d] fp16
            vt = [vvp.tile([128, D], fp16, tag=f"v{st}", name=f"v{st}") for st in range(ST)]
            # attnT[ss]: [128 s-part, 1024 q] fp16
            att = [atp.tile([128, NQ], fp16, tag=f"att{ss}", name=f"att{ss}") for ss in range(ST)]

            def load_stile(st, v_early=True):
                xt = loadp.tile([128, D], fp32, tag="xt", name=f"xt{st}")
                nc.sync.dma_start(xt[:], xk_d[st * 128:(st + 1) * 128, :])
                if v_early:
                    # V = x * sign(bv_v)  (broadcast tile), fp16 out
                    nc.vector.tensor_mul(vt[st][:], xt[:], bvs_sb[:])
                # S = sign(x), bf16
                sb = loadp.tile([128, D], bf16, tag="sb", name=f"sb{st}")
                nc.scalar.activation(sb[:], xt[:], AF.Sign)
                # transpose into skt[st]: 8 x [128,128] bf16 PE transposes
                # (documented production path: matmul(is_transpose) via an
                # identity moving operand, PSUM out, DVE copy back to SBUF).
                # Keeps the serialized HWDGE DMA ring out of the critical path.
                for dk in range(DT):
                    pt = ptp.tile([128, 128], bf16, tag="pt",
                                  name=f"pt{st}_{dk}")
                    nc.tensor.transpose(pt[:], sb[:, dk * 128:(dk + 1) * 128],
                                        ident_sb[:])
                    nc.vector.tensor_copy(skt[st][:, dk * 128:(dk + 1) * 128],
                                          pt[:])
                return xt

            def build_scq(g):
                # q cols of group g live in s-tile 4g (canonical chunk first half
                # = canonical rows [512g, 512g+256) = s-tiles 4g, 4g+1)
                for dk in range(DT):
                    # cols 0..127 from skt[4g], 128..255 from skt[4g+1]
                    nc.vector.tensor_scalar_mul(
                        scq[g][:, dk * 256:dk * 256 + 128],
                        skt[4 * g][:, dk * 128:(dk + 1) * 128],
                        cmat_sb[:, dk:dk + 1],
                    )
                    nc.vector.tensor_scalar_mul(
                        scq[g][:, dk * 256 + 128:dk * 256 + 256],
                        skt[4 * g + 1][:, dk * 128:(dk + 1) * 128],
                        cmat_sb[:, dk:dk + 1],
                    )

            def scores(ss):
                """scoresT tile rows s=[128ss,128ss+128) x q col groups g0..3.

                dk is the outer loop so the stationary operand (skt slice) is
                reused across the g-groups: 1 LDWEIGHTS per (ss, dk) instead
                of one per matmul.
                """
                g0 = ss // 4
                wq = ss % 4
                for g in range(g0, NG):
                    ps = psp.tile([128, 256], fp32, tag="ps", name=f"ps{ss}_{g}")
                    for dk in range(DT):
                        nc.tensor.matmul(
                            ps[:],
                            skt[ss][:, dk * 128:(dk + 1) * 128],
                            scq[g][:, dk * 256:(dk + 1) * 256],
                            start=(dk == 0),
                            stop=(dk == DT - 1),
                        )
                    dst = att[ss][:, g * 256:(g + 1) * 256]
                    # attn = sigmoid(scores/32 * 4)
                    nc.scalar.activation(dst, ps[:], AF.Sigmoid, scale=0.125)
                    if g == g0:
                        # boundary chunk: apply causal mask
                        nc.vector.tensor_mul(dst, dst, mask_sb[wq][:])

            def av(ts):
                """output rows t=[128ts,128ts+128): accumulate over s prefix."""
                j = ts // 2
                nss = 4 * (j + 1)
                ob = outp.tile([128, D], fp32, tag="ob", name=f"ob{ts}")
                for dh in range(2):
                    po = pop.tile([128, 512], fp32, tag="po", name=f"po{ts}_{dh}")
                    for ss in range(nss):
                        nc.tensor.matmul(
                            po[:],
                            att[ss][:, ts * 128:(ts + 1) * 128],
                            vt[ss][:, dh * 512:(dh + 1) * 512],
                            start=(ss == 0),
                            stop=(ss == nss - 1),
                        )
                    nc.vector.tensor_copy(ob[:, dh * 512:(dh + 1) * 512], po[:])
                nc.scalar.dma_start(out_d[ts * 128:(ts + 1) * 128, :], ob[:])

            # ---- emission order ----
            # q-source pairs (4g, 4g+1) descending g so scq[g..3] exist when
            # scores(ss) needs them; second-half pairs (4g+2, 4g+3) descending
            # interleaved to keep the PE fed while the next q-pair loads.
            # AV(ts) is emitted once att[0..4j+3] are complete.
            def pair_a(g):
                load_stile(4 * g)
                load_stile(4 * g + 1)
                build_scq(g)
                scores(4 * g)
                scores(4 * g + 1)

            def pair_b(g):
                load_stile(4 * g + 2)
                load_stile(4 * g + 3)
                scores(4 * g + 2)
                scores(4 * g + 3)

            for g in [3, 2, 1, 0]:
                pair_a(g)
            for g in [0, 1, 2, 3]:
                pair_b(g)
                av(2 * g)
                av(2 * g + 1)

    nc.compile()
    return nc


def host_inputs(x, bv_q, bv_k, bv_v):
    """Build per-core input maps (all host work is O(small) or a copy)."""
    x = np.ascontiguousarray(np.asarray(x, dtype=np.float32))
    sq = np.sign(np.asarray(bv_q, dtype=np.float32))
    sk = np.sign(np.asarray(bv_k, dtype=np.float32))
    sv = np.sign(np.asarray(bv_v, dtype=np.float32))
    c = (sq * sk).astype(np.float32)                     # [D]
    cmat = np.ascontiguousarray(c.reshape(DT, 128).T)    # [128, DT]
    bvs = np.ascontiguousarray(np.broadcast_to(sv, (128, D)))

    ident = np.ascontiguousarray(np.eye(128, dtype=np.float32)).astype(
        __import__("ml_dtypes").bfloat16)
    masks = {}
    for parity in (0, 1):
        m = np.zeros((4, 128, 256), np.float16)
        wo = np.arange(512)[:, None]                     # boundary s offset
        ct = np.arange(256)[None, :]                     # q col offset in group
        if parity == 0:
            keep = wo <= ct                              # orig offsets equal
        else:
            so = np.where(wo < 256, wo + 256, wo - 256)  # swapped halves
            keep = so <= ct + 256
        masks[parity] = np.ascontiguousarray(
            keep.astype(np.float16).reshape(4, 128, 256))

    in_maps = []
    for core in range(NCORES):
        b, parity = core // 2, core % 2
        xb = x[b]
        if parity == 0:
            xkc = xb
        else:
            xkc = np.ascontiguousarray(
                xb.reshape(NG, 2, 256, D)[:, ::-1].reshape(T, D))
        in_maps.append({
            "xk": xkc,
            "cmat": cmat,
            "bvs": bvs,
            "maskt": masks[parity],
            "ident": ident,
        })
    return in_maps


def assemble_output(results):
    out = np.zeros((B, T, D), np.float32)
    for core in range(NCORES):
        b, parity = core // 2, core % 2
        o = np.asarray(results[core]["out"], dtype=np.float32).reshape(NG, 256, D)
        for j in range(NG):
            r0 = 512 * j + 256 * parity
            out[b, r0:r0 + 256] = o[j]
    return out


def kernel(x, bv_q, bv_k, bv_v):
    from concourse.bass_utils import run_bass_kernel_spmd

    if "nc" not in _CACHE:
        _CACHE["nc"] = build_nc()
    nc = _CACHE["nc"]

    in_maps = host_inputs(x, bv_q, bv_k, bv_v)
    res = run_bass_kernel_spmd(nc, in_maps, list(range(NCORES)))
    _CACHE["last_result"] = res
    return assemble_output(res.results)



# revision 2
# speedup vs baseline: 1.0998x; 1.0998x over previous
"""HDC binary attention kernel for 8 trn2 NeuronCores.

Problem: B,T,D = 4,2048,1024
    Q = sign(x * sign(bv_q)); K = sign(x * sign(bv_k)); V = x * sign(bv_v)
    scores = (Q @ K^T) / sqrt(D), causal
    out = sigmoid(4*scores) * causal_mask @ V

Math used by the kernel:
    sign(x*bq) = sign(x)*sign(bq) elementwise, so with S = sign(x) (+-1) and
    c[d] = sign(bv_q)[d]*sign(bv_k)[d]:
        scores[t,s] = sum_d S[t,d]*c[d]*S[s,d] / 32
    We compute scores TRANSPOSED (s on partitions) via
        scoresT = SkT.T @ (c * SqT)   (contraction d on partitions, bf16 exact)
    then attnT = sigmoid(scoresT * 0.125) (* mask on diagonal chunks), fp16,
    and out = attnT.T @ V accumulated over s-subtiles (fp16 matmul).

The host supplies x in BOTH layouts as bf16 (sign(bf16(x)) == sign(x), and
bf16 V is well within the error budget): natural [T,D] for the V path and
transposed [D,T] for the S^T path. This removes all on-device transposes;
S^T tiles are produced directly by ACT Sign on the [d-part, s] tiles, and
the c-scaled Q^T copy is ACT Sign with a per-partition scale (c = +-1).

Sharding: 2 cores per batch. Each 512-row chunk of T is split in half:
    core parity 0 takes rows [512j, 512j+256), parity 1 takes [512j+256, 512j+512).
For SPMD uniformity the host permutes K/V rows for parity-1 cores (swapping the
halves of every 512-chunk) so that each core's q rows always sit at canonical
positions [512j, 512j+256); causal boundary handling is via host-built masks.
Each q group j attends to canonical s < 512*(j+1); full 512-chunks below the
boundary are permutation-invariant, the boundary chunk is masked explicitly.
"""

import numpy as np

B, T, D = 4, 2048, 1024
NQ = 1024          # q rows per core
NCORES = 8
ST = 16            # s-tiles of 128 rows
DT = 8             # d-tiles of 128
NG = 4             # q groups of 256 rows per core

_CACHE = {}


def build_nc():
    """Build + schedule + compile the (single, SPMD-uniform) bass program."""
    import concourse.bass as bass
    import concourse.bacc as bacc
    import concourse.mybir as mybir
    import concourse.tile as tile

    fp32 = mybir.dt.float32
    bf16 = mybir.dt.bfloat16
    fp16 = mybir.dt.float16
    AF = mybir.ActivationFunctionType

    nc = bacc.Bacc("TRN2", target_bir_lowering=False, debug=False)

    xt_d = nc.dram_tensor("xt", [D, T], bf16, kind="ExternalInput").ap()
    xn_d = nc.dram_tensor("xn", [T, D], bf16, kind="ExternalInput").ap()
    cmat_d = nc.dram_tensor("cmat", [128, DT], fp32, kind="ExternalInput").ap()
    bvs_d = nc.dram_tensor("bvs", [128, D], bf16, kind="ExternalInput").ap()
    # maskt[wq][p, ct]: keep for boundary s-offset (128*wq+p) vs q col offset ct
    mask_d = nc.dram_tensor("maskt", [4, 128, 256], fp16, kind="ExternalInput").ap()
    out_d = nc.dram_tensor("out", [NQ, D], fp32, kind="ExternalOutput").ap()

    with tile.TileContext(nc) as tc:
        with (
            tc.tile_pool(name="const", bufs=1) as constp,
            tc.tile_pool(name="load", bufs=6) as loadp,
            tc.tile_pool(name="vload", bufs=4) as vloadp,
            tc.tile_pool(name="kt", bufs=1) as ktp,
            tc.tile_pool(name="qt", bufs=1) as qtp,
            tc.tile_pool(name="vv", bufs=1) as vvp,
            tc.tile_pool(name="at", bufs=1) as atp,
            tc.tile_pool(name="ps", bufs=3, space="PSUM") as psp,
            tc.tile_pool(name="po", bufs=3, space="PSUM") as pop,
            tc.tile_pool(name="outb", bufs=3) as outp,
        ):
            # ---- constants ----
            bvs_sb = constp.tile([128, D], bf16, tag="bvs")
            nc.gpsimd.dma_start(bvs_sb[:], bvs_d)
            cmat_sb = constp.tile([128, DT], fp32, tag="cmat")
            nc.gpsimd.dma_start(cmat_sb[:], cmat_d)
            mask_sb = [constp.tile([128, 256], fp16, tag=f"mask{w}", name=f"mask{w}") for w in range(4)]
            for w in range(4):
                nc.gpsimd.dma_start(mask_sb[w][:], mask_d[w])

            # ---- persistent arrays ----
            # sktd[dk][q]: [128 d-part, 512] bf16 = sign(x)^T for d-tile dk,
            # s columns [512q, 512q+512)
            sktd = [[ktp.tile([128, 512], bf16, tag=f"skt{dk}_{q}",
                              name=f"skt{dk}_{q}") for q in range(4)]
                    for dk in range(DT)]
            # scq[g]: [128 d-part, 8*256] bf16; cols dk*256+ct = c*S^T at
            # q col (256g+ct)
            scq = [qtp.tile([128, DT * 256], bf16, tag=f"scq{g}", name=f"scq{g}")
                   for g in range(NG)]
            # V[st]: [128 s-part, 1024 d] fp16
            vt = [vvp.tile([128, D], fp16, tag=f"v{st}", name=f"v{st}")
                  for st in range(ST)]
            # attnT[ss]: [128 s-part, 1024 q] fp16
            att = [atp.tile([128, NQ], fp16, tag=f"att{ss}", name=f"att{ss}")
                   for ss in range(ST)]

            def load_block(q):
                """DMA + sign the 8 d-tiles of s-block q; build scq[q]."""
                for dk in range(DT):
                    xtt = loadp.tile([128, 512], bf16, tag="xtt",
                                     name=f"xtt{dk}_{q}")
                    nc.sync.dma_start(
                        xtt[:], xt_d[dk * 128:(dk + 1) * 128,
                                     q * 512:(q + 1) * 512])
                    nc.scalar.activation(sktd[dk][q][:], xtt[:], AF.Sign)
                    # q-cols of group g=q are the first 256 cols of this block
                    nc.scalar.activation(
                        scq[q][:, dk * 256:(dk + 1) * 256],
                        xtt[:, 0:256], AF.Sign,
                        scale=cmat_sb[:, dk:dk + 1])

            def load_v(st):
                xnt = vloadp.tile([128, D], bf16, tag="xnt", name=f"xnt{st}")
                nc.gpsimd.dma_start(xnt[:], xn_d[st * 128:(st + 1) * 128, :])
                nc.vector.tensor_mul(vt[st][:], xnt[:], bvs_sb[:])

            def scores(ss):
                """scoresT tile rows s=[128ss,128ss+128) x q col groups g0..3."""
                g0 = ss // 4
                wq = ss % 4
                co = (ss % 4) * 128
                for g in range(g0, NG):
                    ps = psp.tile([128, 256], fp32, tag="ps", name=f"ps{ss}_{g}")
                    for dk in range(DT):
                        nc.tensor.matmul(
                            ps[:],
                            sktd[dk][g0][:, co:co + 128],
                            scq[g][:, dk * 256:(dk + 1) * 256],
                            start=(dk == 0),
                            stop=(dk == DT - 1),
                        )
                    dst = att[ss][:, g * 256:(g + 1) * 256]
                    # attn = sigmoid(scores/32 * 4)
                    nc.scalar.activation(dst, ps[:], AF.Sigmoid, scale=0.125)
                    if g == g0:
                        # boundary chunk: apply causal mask
                        nc.vector.tensor_mul(dst, dst, mask_sb[wq][:])

            def av(ts):
                """output rows t=[128ts,128ts+128): accumulate over s prefix."""
                j = ts // 2
                nss = 4 * (j + 1)
                ob = outp.tile([128, D], fp32, tag="ob", name=f"ob{ts}")
                for dh in range(2):
                    po = pop.tile([128, 512], fp32, tag="po", name=f"po{ts}_{dh}")
                    for ss in range(nss):
                        nc.tensor.matmul(
                            po[:],
                            att[ss][:, ts * 128:(ts + 1) * 128],
                            vt[ss][:, dh * 512:(dh + 1) * 512],
                            start=(ss == 0),
                            stop=(ss == nss - 1),
                        )
                    nc.vector.tensor_copy(ob[:, dh * 512:(dh + 1) * 512], po[:])
                nc.scalar.dma_start(out_d[ts * 128:(ts + 1) * 128, :], ob[:])

            # ---- emission order ----
            # s-blocks descending q so block 3 (needed only by g=3 scores)
            # lands first and scores start after ~8 sub-tile DMAs; each block
            # q enables scores(ss in 4q..4q+3, g >= q). V loads interleave on
            # the gpsimd queue. AV(ts) only completes after block 0, so all
            # AV is emitted last.
            for q in [3, 2, 1, 0]:
                load_block(q)
                for st in range(4 * q, 4 * q + 4):
                    load_v(st)
                for ss in range(4 * q, 4 * q + 4):
                    scores(ss)
            for ts in range(8):
                av(ts)

    nc.compile()
    return nc


def host_inputs(x, bv_q, bv_k, bv_v):
    """Build per-core input maps (all host work is a cast/copy or O(small))."""
    import ml_dtypes
    bfloat16 = ml_dtypes.bfloat16

    x = np.asarray(x, dtype=np.float32)
    sq = np.sign(np.asarray(bv_q, dtype=np.float32))
    sk = np.sign(np.asarray(bv_k, dtype=np.float32))
    sv = np.sign(np.asarray(bv_v, dtype=np.float32))
    c = (sq * sk).astype(np.float32)                     # [D]
    cmat = np.ascontiguousarray(c.reshape(DT, 128).T)    # [128, DT]
    bvs = np.ascontiguousarray(
        np.broadcast_to(sv, (128, D))).astype(bfloat16)

    masks = {}
    for parity in (0, 1):
        wo = np.arange(512)[:, None]                     # boundary s offset
        ct = np.arange(256)[None, :]                     # q col offset in group
        if parity == 0:
            keep = wo <= ct                              # orig offsets equal
        else:
            so = np.where(wo < 256, wo + 256, wo - 256)  # swapped halves
            keep = so <= ct + 256
        masks[parity] = np.ascontiguousarray(
            keep.astype(np.float16).reshape(4, 128, 256))

    in_maps = []
    for core in range(NCORES):
        b, parity = core // 2, core % 2
        xb = x[b]
        if parity == 0:
            xkc = xb
        else:
            xkc = xb.reshape(NG, 2, 256, D)[:, ::-1].reshape(T, D)
        xn = np.ascontiguousarray(xkc).astype(bfloat16)
        xt = np.ascontiguousarray(xkc.T).astype(bfloat16)
        in_maps.append({
            "xt": xt,
            "xn": xn,
            "cmat": cmat,
            "bvs": bvs,
            "maskt": masks[parity],
        })
    return in_maps


def assemble_output(results):
    out = np.zeros((B, T, D), np.float32)
    for core in range(NCORES):
        b, parity = core // 2, core % 2
        o = np.asarray(results[core]["out"], dtype=np.float32).reshape(NG, 256, D)
        for j in range(NG):
            r0 = 512 * j + 256 * parity
            out[b, r0:r0 + 256] = o[j]
    return out


def kernel(x, bv_q, bv_k, bv_v):
    from concourse.bass_utils import run_bass_kernel_spmd

    if "nc" not in _CACHE:
        _CACHE["nc"] = build_nc()
    nc = _CACHE["nc"]

    in_maps = host_inputs(x, bv_q, bv_k, bv_v)
    res = run_bass_kernel_spmd(nc, in_maps, list(range(NCORES)))
    _CACHE["last_result"] = res
    return assemble_output(res.results)


# revision 4
# speedup vs baseline: 1.3589x; 1.2356x over previous
"""HDC binary attention kernel for 8 trn2 NeuronCores.

Problem: B,T,D = 4,2048,1024
    Q = sign(x * sign(bv_q)); K = sign(x * sign(bv_k)); V = x * sign(bv_v)
    scores = (Q @ K^T) / sqrt(D), causal
    out = sigmoid(4*scores) * causal_mask @ V

Math used by the kernel:
    sign(x*bq) = sign(x)*sign(bq) elementwise, so with S = sign(x) (+-1) and
    c[d] = sign(bv_q)[d]*sign(bv_k)[d]:
        scores[t,s] = sum_d S[t,d]*c[d]*S[s,d] / 32
    We compute scores TRANSPOSED (s on partitions) via
        scoresT = SkT.T @ (c * SqT)   (contraction d on partitions)
    then attnT = sigmoid(scoresT * 0.125) (* mask on diagonal chunks), fp16,
    and out = attnT.T @ V accumulated over s-subtiles (fp16 matmul).

Precision/layout choices:
  - Host supplies x in BOTH layouts as bf16 (sign(bf16(x)) == sign(x), and
    bf16 V is well within the error budget): natural [T,D] for the V path
    and transposed [D,T] for the S^T path. No on-device transposes.
  - S^T and c*S^T are +-1, stored as fp8e4 (exact); the score matmuls run
    in fp8 DoubleRow mode (256-deep contraction per matmul, 0.5 cyc/row).
    Products are +-1 and accumulate in fp32 PSUM, so scores are exact.
  - attn and V are fp16; output fp32.

Sharding: 2 cores per batch. Each 512-row chunk of T is split in half:
    core parity 0 takes rows [512j, 512j+256), parity 1 takes [512j+256, 512j+512).
For SPMD uniformity the host permutes K/V rows for parity-1 cores (swapping the
halves of every 512-chunk) so that each core's q rows always sit at canonical
positions [512j, 512j+256); causal boundary handling is via host-built masks.
Each q group j attends to canonical s < 512*(j+1); full 512-chunks below the
boundary are permutation-invariant, the boundary chunk is masked explicitly.
"""

import numpy as np

B, T, D = 4, 2048, 1024
NQ = 1024          # q rows per core
NCORES = 8
ST = 16            # s-tiles of 128 rows
DT = 8             # d-tiles of 128
NG = 4             # q groups of 256 rows per core

_CACHE = {}


def build_nc():
    """Build + schedule + compile the (single, SPMD-uniform) bass program."""
    import concourse.bass as bass
    import concourse.bacc as bacc
    import concourse.mybir as mybir
    import concourse.tile as tile

    fp32 = mybir.dt.float32
    bf16 = mybir.dt.bfloat16
    fp16 = mybir.dt.float16
    fp8 = mybir.dt.float8e4
    AF = mybir.ActivationFunctionType
    DR = mybir.MatmulPerfMode.DoubleRow

    nc = bacc.Bacc("TRN2", target_bir_lowering=False, debug=False)

    xt_d = nc.dram_tensor("xt", [D, T], bf16, kind="ExternalInput").ap()
    xn_d = nc.dram_tensor("xn", [T, D], bf16, kind="ExternalInput").ap()
    cmat_d = nc.dram_tensor("cmat", [128, DT], fp32, kind="ExternalInput").ap()
    bvs_d = nc.dram_tensor("bvs", [128, D], bf16, kind="ExternalInput").ap()
    # maskt[wq][p, ct]: keep for boundary s-offset (128*wq+p) vs q col offset ct
    mask_d = nc.dram_tensor("maskt", [4, 128, 256], fp16, kind="ExternalInput").ap()
    out_d = nc.dram_tensor("out", [NQ, D], fp32, kind="ExternalOutput").ap()

    with tile.TileContext(nc) as tc:
        with (
            tc.tile_pool(name="const", bufs=1) as constp,
            tc.tile_pool(name="load", bufs=6) as loadp,
            tc.tile_pool(name="vload", bufs=4) as vloadp,
            tc.tile_pool(name="kt", bufs=1) as ktp,
            tc.tile_pool(name="qt", bufs=1) as qtp,
            tc.tile_pool(name="vv", bufs=1) as vvp,
            tc.tile_pool(name="at", bufs=1) as atp,
            tc.tile_pool(name="ps", bufs=3, space="PSUM") as psp,
            tc.tile_pool(name="po", bufs=3, space="PSUM") as pop,
            tc.tile_pool(name="outb", bufs=3) as outp,
        ):
            # ---- constants (sync queue so they land before the xt stream) ----
            bvs_sb = constp.tile([128, D], bf16, tag="bvs")
            nc.sync.dma_start(bvs_sb[:], bvs_d)
            cmat_sb = constp.tile([128, DT], fp32, tag="cmat")
            nc.sync.dma_start(cmat_sb[:], cmat_d)
            mask_sb = [constp.tile([128, 256], fp16, tag=f"mask{w}", name=f"mask{w}") for w in range(4)]
            for w in range(4):
                nc.sync.dma_start(mask_sb[w][:], mask_d[w])

            # ---- persistent arrays ----
            # skt8[q]: [128 d-part, dk, 512] fp8 = sign(x)^T for s block q;
            # 3D so DoubleRow matmuls can take [:, 2e:2e+2, cols] slices.
            skt8 = [ktp.tile([128, DT, 512], fp8, tag=f"skt{q}", name=f"skt{q}")
                    for q in range(4)]
            # scq8: [128 d-part, dk, 1024] fp8; [:, dk, 256g+ct] = c*S^T at
            # q col (256g+ct)
            scq8 = qtp.tile([128, DT, NQ], fp8, tag="scq")
            # V[st]: [128 s-part, 1024 d] fp16
            vt = [vvp.tile([128, D], fp16, tag=f"v{st}", name=f"v{st}")
                  for st in range(ST)]
            # attnT[ss]: [128 s-part, 1024 q] fp16
            att = [atp.tile([128, NQ], fp16, tag=f"att{ss}", name=f"att{ss}")
                   for ss in range(ST)]

            def load_block(q):
                """DMA + sign the 8 d-tiles of s-block q; build scq8[:, :, q]."""
                for dk in range(DT):
                    xtt = loadp.tile([128, 512], bf16, tag="xtt",
                                     name=f"xtt{dk}_{q}")
                    nc.sync.dma_start(
                        xtt[:], xt_d[dk * 128:(dk + 1) * 128,
                                     q * 512:(q + 1) * 512])
                    nc.scalar.activation(skt8[q][:, dk, :], xtt[:], AF.Sign)
                    # q-cols of group g=q are the first 256 cols of this block
                    nc.scalar.activation(
                        scq8[:, dk, q * 256:(q + 1) * 256],
                        xtt[:, 0:256], AF.Sign,
                        scale=cmat_sb[:, dk:dk + 1])

            def load_v(st):
                xnt = vloadp.tile([128, D], bf16, tag="xnt", name=f"xnt{st}")
                nc.scalar.dma_start(xnt[:], xn_d[st * 128:(st + 1) * 128, :])
                nc.vector.tensor_mul(vt[st][:], xnt[:], bvs_sb[:])

            def score_unit(ss, g):
                """scoresT rows s=[128ss,128ss+128) x q cols [256g, 256g+256)."""
                qb = ss // 4
                wq = ss % 4
                co = wq * 128
                ps = psp.tile([128, 256], fp32, tag="ps", name=f"ps{ss}_{g}")
                for e in range(DT // 2):
                    nc.tensor.matmul(
                        ps[:],
                        skt8[qb][:, 2 * e:2 * e + 2, co:co + 128],
                        scq8[:, 2 * e:2 * e + 2, g * 256:(g + 1) * 256],
                        start=(e == 0),
                        stop=(e == DT // 2 - 1),
                        perf_mode=DR,
                    )
                dst = att[ss][:, g * 256:(g + 1) * 256]
                # attn = sigmoid(scores/32 * 4)
                nc.scalar.activation(dst, ps[:], AF.Sigmoid, scale=0.125)
                if g == qb:
                    # boundary chunk: apply causal mask
                    nc.vector.tensor_mul(dst, dst, mask_sb[wq][:])

            def av(ts):
                """output rows t=[128ts,128ts+128): accumulate over s prefix."""
                j = ts // 2
                nss = 4 * (j + 1)
                ob = outp.tile([128, D], fp32, tag="ob", name=f"ob{ts}")
                for dh in range(2):
                    po = pop.tile([128, 512], fp32, tag="po", name=f"po{ts}_{dh}")
                    for ss in range(nss):
                        nc.tensor.matmul(
                            po[:],
                            att[ss][:, ts * 128:(ts + 1) * 128],
                            vt[ss][:, dh * 512:(dh + 1) * 512],
                            start=(ss == 0),
                            stop=(ss == nss - 1),
                        )
                    nc.vector.tensor_copy(ob[:, dh * 512:(dh + 1) * 512], po[:])
                    nc.gpsimd.dma_start(
                        out_d[ts * 128:(ts + 1) * 128, dh * 512:(dh + 1) * 512],
                        ob[:, dh * 512:(dh + 1) * 512])

            # ---- emission order ----
            # Ascending s-blocks. Stage q: load block q (building scq for
            # group g=q), then score all s-tiles ss <= 4q+3 against q-group
            # g=q (their skt blocks are already resident), then the two AV
            # row-tiles of group q, which depend only on those scores. This
            # overlaps AV matmuls with later block DMA instead of
            # serializing all AV at the tail.
            for q in range(4):
                load_block(q)
                for st in range(4 * q, 4 * q + 4):
                    load_v(st)
                for ss in range(4 * q + 4):
                    score_unit(ss, q)
                av(2 * q)
                av(2 * q + 1)

    nc.compile()
    return nc


def host_inputs(x, bv_q, bv_k, bv_v):
    """Build per-core input maps (all host work is a cast/copy or O(small))."""
    import ml_dtypes
    bfloat16 = ml_dtypes.bfloat16

    x = np.asarray(x, dtype=np.float32)
    sq = np.sign(np.asarray(bv_q, dtype=np.float32))
    sk = np.sign(np.asarray(bv_k, dtype=np.float32))
    sv = np.sign(np.asarray(bv_v, dtype=np.float32))
    c = (sq * sk).astype(np.float32)                     # [D]
    cmat = np.ascontiguousarray(c.reshape(DT, 128).T)    # [128, DT]
    bvs = np.ascontiguousarray(
        np.broadcast_to(sv, (128, D))).astype(bfloat16)

    masks = {}
    for parity in (0, 1):
        wo = np.arange(512)[:, None]                     # boundary s offset
        ct = np.arange(256)[None, :]                     # q col offset in group
        if parity == 0:
            keep = wo <= ct                              # orig offsets equal
        else:
            so = np.where(wo < 256, wo + 256, wo - 256)  # swapped halves
            keep = so <= ct + 256
        masks[parity] = np.ascontiguousarray(
            keep.astype(np.float16).reshape(4, 128, 256))

    in_maps = []
    for core in range(NCORES):
        b, parity = core // 2, core % 2
        xb = x[b]
        if parity == 0:
            xkc = xb
        else:
            xkc = xb.reshape(NG, 2, 256, D)[:, ::-1].reshape(T, D)
        xn = np.ascontiguousarray(xkc).astype(bfloat16)
        xt = np.ascontiguousarray(xkc.T).astype(bfloat16)
        in_maps.append({
            "xt": xt,
            "xn": xn,
            "cmat": cmat,
            "bvs": bvs,
            "maskt": masks[parity],
        })
    return in_maps


def assemble_output(results):
    out = np.zeros((B, T, D), np.float32)
    for core in range(NCORES):
        b, parity = core // 2, core % 2
        o = np.asarray(results[core]["out"], dtype=np.float32).reshape(NG, 256, D)
        for j in range(NG):
            r0 = 512 * j + 256 * parity
            out[b, r0:r0 + 256] = o[j]
    return out


def kernel(x, bv_q, bv_k, bv_v):
    from concourse.bass_utils import run_bass_kernel_spmd

    if "nc" not in _CACHE:
        _CACHE["nc"] = build_nc()
    nc = _CACHE["nc"]

    in_maps = host_inputs(x, bv_q, bv_k, bv_v)
    res = run_bass_kernel_spmd(nc, in_maps, list(range(NCORES)))
    _CACHE["last_result"] = res
    return assemble_output(res.results)


# revision 5
# speedup vs baseline: 1.4416x; 1.0608x over previous
"""HDC binary attention kernel for 8 trn2 NeuronCores.

Problem: B,T,D = 4,2048,1024
    Q = sign(x * sign(bv_q)); K = sign(x * sign(bv_k)); V = x * sign(bv_v)
    scores = (Q @ K^T) / sqrt(D), causal
    out = sigmoid(4*scores) * causal_mask @ V

Math used by the kernel:
    sign(x*bq) = sign(x)*sign(bq) elementwise, so with S = sign(x) (+-1) and
    c[d] = sign(bv_q)[d]*sign(bv_k)[d]:
        scores[t,s] = sum_d S[t,d]*c[d]*S[s,d] / 32
    We compute scores TRANSPOSED (s on partitions) via
        scoresT = SkT.T @ (c * SqT)   (contraction d on partitions)
    then attnT = sigmoid(scoresT * 0.125) (* mask on diagonal chunks), fp16,
    and out = attnT.T @ V accumulated over s-subtiles (fp16 matmul).

Precision/layout choices:
  - Host supplies x in BOTH layouts as bf16 (sign(bf16(x)) == sign(x), and
    bf16 V is well within the error budget): a [dk,s]-tiled transposed
    layout for the S^T path and an s-tiled natural layout for the V path.
    No on-device transposes.
  - S^T and c*S^T are +-1, stored as fp8e4 (exact); the score matmuls run
    in fp8 DoubleRow mode (256-deep contraction per matmul, 0.5 cyc/row).
    Products are +-1 and accumulate in fp32 PSUM, so scores are exact.
  - attn and V are fp16; output fp16 (upcast on host).
  - DMAs are coarse (one per 512-col s-block / 512-row s-group) to bound
    HWDGE descriptor-generation occupancy; the first block is split in 4 so
    the PE can start early.

Sharding: 2 cores per batch. Each 512-row chunk of T is split in half:
    core parity 0 takes rows [512j, 512j+256), parity 1 takes [512j+256, 512j+512).
For SPMD uniformity the host permutes K/V rows for parity-1 cores (swapping the
halves of every 512-chunk) so that each core's q rows always sit at canonical
positions [512j, 512j+256); causal boundary handling is via host-built masks.
Each q group j attends to canonical s < 512*(j+1); full 512-chunks below the
boundary are permutation-invariant, the boundary chunk is masked explicitly.
"""

import numpy as np

B, T, D = 4, 2048, 1024
NQ = 1024          # q rows per core
NCORES = 8
ST = 16            # s-tiles of 128 rows
DT = 8             # d-tiles of 128
NG = 4             # q groups of 256 rows per core

_CACHE = {}


def build_nc():
    """Build + schedule + compile the (single, SPMD-uniform) bass program."""
    import concourse.bass as bass
    import concourse.bacc as bacc
    import concourse.mybir as mybir
    import concourse.tile as tile

    fp32 = mybir.dt.float32
    bf16 = mybir.dt.bfloat16
    fp16 = mybir.dt.float16
    fp8 = mybir.dt.float8e4
    AF = mybir.ActivationFunctionType
    DR = mybir.MatmulPerfMode.DoubleRow

    nc = bacc.Bacc("TRN2", target_bir_lowering=False, debug=False)

    # xtb[q, p, dk, s]: x^T[128dk+p, 512q+s]  (bf16, one DMA per s-block)
    xtb_d = nc.dram_tensor("xtb", [4, 128, DT * 512], bf16, kind="ExternalInput").ap()
    # xnb[i, p, j, d]: x[512i+128j+p, d]      (bf16, one DMA per s-group)
    xnb_d = nc.dram_tensor("xnb", [4, 128, 4 * D], bf16, kind="ExternalInput").ap()
    # cbig[p, dk, j] = c[128dk+p]  (fp8, +-1)
    cbig_d = nc.dram_tensor("cbig", [128, DT * 256], fp8, kind="ExternalInput").ap()
    bvs_d = nc.dram_tensor("bvs", [128, D], bf16, kind="ExternalInput").ap()
    # maskp[p, 256*wq+ct]: keep for boundary s-offset (128*wq+p) vs q col ct
    mask_d = nc.dram_tensor("maskp", [128, 4 * 256], fp16, kind="ExternalInput").ap()
    out_d = nc.dram_tensor("out", [NQ, D], fp16, kind="ExternalOutput").ap()

    with tile.TileContext(nc) as tc:
        with (
            tc.tile_pool(name="const", bufs=1) as constp,
            tc.tile_pool(name="load", bufs=2) as loadp,
            tc.tile_pool(name="vload", bufs=2) as vloadp,
            tc.tile_pool(name="kt", bufs=1) as ktp,
            tc.tile_pool(name="qt", bufs=1) as qtp,
            tc.tile_pool(name="vv", bufs=1) as vvp,
            tc.tile_pool(name="at", bufs=1) as atp,
            tc.tile_pool(name="ps", bufs=3, space="PSUM") as psp,
            tc.tile_pool(name="po", bufs=3, space="PSUM") as pop,
            tc.tile_pool(name="outb", bufs=3) as outp,
        ):
            # ---- constants (sync queue so they land before the xt stream) ----
            cbig_sb = constp.tile([128, DT, 256], fp8, tag="cbig")
            nc.sync.dma_start(cbig_sb[:], cbig_d.rearrange("p (dk j) -> p dk j", dk=DT))
            mask_sb = constp.tile([128, 4 * 256], fp16, tag="maskp")
            nc.sync.dma_start(mask_sb[:], mask_d)
            bvs_sb = constp.tile([128, D], bf16, tag="bvs")
            nc.sync.dma_start(bvs_sb[:], bvs_d)

            # ---- persistent arrays ----
            # skt8[q]: [128 d-part, dk, 512] fp8 = sign(x)^T for s block q;
            # 3D so DoubleRow matmuls can take [:, 2e:2e+2, cols] slices.
            skt8 = [ktp.tile([128, DT, 512], fp8, tag=f"skt{q}", name=f"skt{q}")
                    for q in range(4)]
            # scq8: [128 d-part, dk, 1024] fp8; [:, dk, 256g+ct] = c*S^T at
            # q col (256g+ct)
            scq8 = qtp.tile([128, DT, NQ], fp8, tag="scq")
            # V[i]: [128 s-part, j, 1024 d] fp16 for s-tiles 4i+j
            vt = [vvp.tile([128, 4, D], fp16, tag=f"v{i}", name=f"v{i}")
                  for i in range(4)]
            # attnT[ss]: [128 s-part, 1024 q] fp16
            att = [atp.tile([128, NQ], fp16, tag=f"att{ss}", name=f"att{ss}")
                   for ss in range(ST)]

            def load_block(q, chunks):
                """DMA + sign s-block q; build scq8[:, :, q-cols]."""
                xtt = loadp.tile([128, DT, 512], bf16, tag="xtt", name=f"xtt{q}")
                ne = DT // (2 * chunks)   # dk-pairs per chunk
                for ch in range(chunks):
                    e0 = ch * ne
                    nc.sync.dma_start(
                        xtt[:, 2 * e0:2 * (e0 + ne), :],
                        xtb_d[q][:, 2 * e0 * 512:2 * (e0 + ne) * 512])
                    nc.scalar.activation(skt8[q][:, 2 * e0:2 * (e0 + ne), :],
                                         xtt[:, 2 * e0:2 * (e0 + ne), :], AF.Sign)
                    # q-cols of group g=q are the first 256 cols of this block
                    nc.vector.tensor_mul(
                        scq8[:, 2 * e0:2 * (e0 + ne), q * 256:(q + 1) * 256],
                        skt8[q][:, 2 * e0:2 * (e0 + ne), 0:256],
                        cbig_sb[:, 2 * e0:2 * (e0 + ne), :])

            def load_vblock(i):
                xnt = vloadp.tile([128, 4, D], bf16, tag="xnt", name=f"xnt{i}")
                nc.scalar.dma_start(xnt[:], xnb_d[i])
                for j in range(4):
                    nc.vector.tensor_mul(vt[i][:, j, :], xnt[:, j, :], bvs_sb[:])

            def score_unit(ss, g):
                """scoresT rows s=[128ss,128ss+128) x q cols [256g, 256g+256)."""
                qb = ss // 4
                wq = ss % 4
                co = wq * 128
                ps = psp.tile([128, 256], fp32, tag="ps", name=f"ps{ss}_{g}")
                for e in range(DT // 2):
                    nc.tensor.matmul(
                        ps[:],
                        skt8[qb][:, 2 * e:2 * e + 2, co:co + 128],
                        scq8[:, 2 * e:2 * e + 2, g * 256:(g + 1) * 256],
                        start=(e == 0),
                        stop=(e == DT // 2 - 1),
                        perf_mode=DR,
                    )
                dst = att[ss][:, g * 256:(g + 1) * 256]
                # attn = sigmoid(scores/32 * 4)
                nc.scalar.activation(dst, ps[:], AF.Sigmoid, scale=0.125)
                if g == qb:
                    # boundary chunk: apply causal mask
                    nc.vector.tensor_mul(dst, dst,
                                         mask_sb[:, wq * 256:(wq + 1) * 256])

            def av(ts):
                """output rows t=[128ts,128ts+128): accumulate over s prefix."""
                j = ts // 2
                nss = 4 * (j + 1)
                ob = outp.tile([128, D], fp16, tag="ob", name=f"ob{ts}")
                for dh in range(2):
                    po = pop.tile([128, 512], fp32, tag="po", name=f"po{ts}_{dh}")
                    for ss in range(nss):
                        nc.tensor.matmul(
                            po[:],
                            att[ss][:, ts * 128:(ts + 1) * 128],
                            vt[ss // 4][:, ss % 4, dh * 512:(dh + 1) * 512],
                            start=(ss == 0),
                            stop=(ss == nss - 1),
                        )
                    nc.vector.tensor_copy(ob[:, dh * 512:(dh + 1) * 512], po[:])
                nc.gpsimd.dma_start(out_d[ts * 128:(ts + 1) * 128, :], ob[:])

            # ---- emission order ----
            # Ascending s-blocks. Stage q: load block q (building scq for
            # group g=q), then score all s-tiles ss <= 4q+3 against q-group
            # g=q (their skt blocks are already resident), then the two AV
            # row-tiles of group q, which depend only on those scores. This
            # overlaps AV matmuls with later block DMA instead of
            # serializing all AV at the tail. Block 0 is split into 4
            # DMA/sign chunks so the first scores start early.
            for q in range(4):
                load_block(q, chunks=(4 if q == 0 else 1))
                load_vblock(q)
                for ss in range(4 * q + 4):
                    score_unit(ss, q)
                av(2 * q)
                av(2 * q + 1)

    nc.compile()
    return nc


def host_inputs(x, bv_q, bv_k, bv_v):
    """Build per-core input maps (all host work is a cast/copy or O(small))."""
    import ml_dtypes
    bfloat16 = ml_dtypes.bfloat16
    f8 = ml_dtypes.float8_e4m3fn

    x = np.asarray(x, dtype=np.float32)
    sq = np.sign(np.asarray(bv_q, dtype=np.float32))
    sk = np.sign(np.asarray(bv_k, dtype=np.float32))
    sv = np.sign(np.asarray(bv_v, dtype=np.float32))
    c = (sq * sk).astype(np.float32)                     # [D]
    # cbig[p, dk*256+j] = c[128dk+p]
    cbig = np.ascontiguousarray(
        np.broadcast_to(c.reshape(DT, 128).T[:, :, None],
                        (128, DT, 256)).reshape(128, DT * 256)).astype(f8)
    bvs = np.ascontiguousarray(
        np.broadcast_to(sv, (128, D))).astype(bfloat16)

    masks = {}
    for parity in (0, 1):
        wo = np.arange(512)[:, None]                     # boundary s offset
        ct = np.arange(256)[None, :]                     # q col offset in group
        if parity == 0:
            keep = wo <= ct                              # orig offsets equal
        else:
            so = np.where(wo < 256, wo + 256, wo - 256)  # swapped halves
            keep = so <= ct + 256
        # [wq*128+p, ct] -> [p, wq*256+ct]
        m = keep.astype(np.float16).reshape(4, 128, 256)
        masks[parity] = np.ascontiguousarray(
            m.transpose(1, 0, 2).reshape(128, 4 * 256))

    in_maps = []
    for core in range(NCORES):
        b, parity = core // 2, core % 2
        xb = x[b]
        if parity == 0:
            xkc = xb
        else:
            xkc = xb.reshape(NG, 2, 256, D)[:, ::-1].reshape(T, D)
        # xtb[q, p, dk, s] = xkc[512q+s, 128dk+p]
        xtb = np.ascontiguousarray(
            xkc.reshape(4, 512, DT, 128).transpose(0, 3, 2, 1)
        ).astype(bfloat16).reshape(4, 128, DT * 512)
        # xnb[i, p, j, d] = xkc[512i+128j+p, d]
        xnb = np.ascontiguousarray(
            xkc.reshape(4, 4, 128, D).transpose(0, 2, 1, 3)
        ).astype(bfloat16).reshape(4, 128, 4 * D)
        in_maps.append({
            "xtb": xtb,
            "xnb": xnb,
            "cbig": cbig,
            "bvs": bvs,
            "maskp": masks[parity],
        })
    return in_maps


def assemble_output(results):
    out = np.zeros((B, T, D), np.float32)
    for core in range(NCORES):
        b, parity = core // 2, core % 2
        o = np.asarray(results[core]["out"], dtype=np.float32).reshape(NG, 256, D)
        for j in range(NG):
            r0 = 512 * j + 256 * parity
            out[b, r0:r0 + 256] = o[j]
    return out


def kernel(x, bv_q, bv_k, bv_v):
    from concourse.bass_utils import run_bass_kernel_spmd

    if "nc" not in _CACHE:
        _CACHE["nc"] = build_nc()
    nc = _CACHE["nc"]

    in_maps = host_inputs(x, bv_q, bv_k, bv_v)
    res = run_bass_kernel_spmd(nc, in_maps, list(range(NCORES)))
    _CACHE["last_result"] = res
    return assemble_output(res.results)


# revision 12
# speedup vs baseline: 1.7273x; 1.1982x over previous
"""HDC binary attention kernel for 8 trn2 NeuronCores.

Problem: B,T,D = 4,2048,1024
    Q = sign(x * sign(bv_q)); K = sign(x * sign(bv_k)); V = x * sign(bv_v)
    scores = (Q @ K^T) / sqrt(D), causal
    out = sigmoid(4*scores) * causal_mask @ V

Math used by the kernel:
    sign(x*bq) = sign(x)*sign(bq) elementwise, so with S = sign(x) (+-1) and
    c[d] = sign(bv_q)[d]*sign(bv_k)[d]:
        scores[t,s] = sum_d S[t,d]*c[d]*S[s,d] / 32
    We compute scores TRANSPOSED (s on partitions) via
        scoresT = SkT.T @ (c * SqT)   (contraction d on partitions)
    then attnT = sigmoid(scoresT * 0.125) (* mask on diagonal chunks), fp16,
    and out = attnT.T @ V accumulated over s-subtiles (fp16 matmul).

Precision/layout choices:
  - Host supplies x in BOTH layouts as bf16 (sign(bf16(x)) == sign(x), and
    bf16 V is well within the error budget): a [dk,s]-tiled transposed
    layout for the S^T path and an s-tiled natural layout for the V path.
    No on-device transposes.
  - S^T and c*S^T are +-1, stored as fp8e4 (exact); the score matmuls run
    in fp8 DoubleRow mode (256-deep contraction per matmul, 0.5 cyc/row).
    Products are +-1 and accumulate in fp32 PSUM, so scores are exact.
  - attn and V are fp16; output fp16 (upcast on host).
  - DMAs are coarse (one per 512-col s-block / 512-row s-group) to bound
    HWDGE descriptor-generation occupancy; the first block is split in 4 so
    the PE can start early.

Sharding: 2 cores per batch. Each 512-row chunk of T is split in half:
    core parity 0 takes rows [512j, 512j+256), parity 1 takes [512j+256, 512j+512).
For SPMD uniformity the host permutes K/V rows for parity-1 cores (swapping the
halves of every 512-chunk) so that each core's q rows always sit at canonical
positions [512j, 512j+256); causal boundary handling is via host-built masks.
Each q group j attends to canonical s < 512*(j+1); full 512-chunks below the
boundary are permutation-invariant, the boundary chunk is masked explicitly.
"""

import numpy as np

B, T, D = 4, 2048, 1024
NQ = 1024          # q rows per core
NCORES = 8
ST = 16            # s-tiles of 128 rows
DT = 8             # d-tiles of 128
NG = 4             # q groups of 256 rows per core

_CACHE = {}


def build_nc():
    """Build + schedule + compile the (single, SPMD-uniform) bass program."""
    import concourse.bass as bass
    import concourse.bacc as bacc
    import concourse.mybir as mybir
    import concourse.tile as tile

    fp32 = mybir.dt.float32
    bf16 = mybir.dt.bfloat16
    fp16 = mybir.dt.float16
    fp8 = mybir.dt.float8e4
    AF = mybir.ActivationFunctionType
    DR = mybir.MatmulPerfMode.DoubleRow

    nc = bacc.Bacc("TRN2", target_bir_lowering=False, debug=False)

    # xtb[q, p, dk, s]: x^T[128dk+p, 512q+s]  (bf16, one DMA per s-block)
    xtb_d = nc.dram_tensor("xtb", [4, 128, DT * 512], bf16, kind="ExternalInput").ap()
    # xnb[i, p, j, d]: x[512i+128j+p, d]      (bf16, one DMA per s-group)
    xnb_d = nc.dram_tensor("xnb", [4, 128, 4 * D], bf16, kind="ExternalInput").ap()
    # cbig[p, dk, j] = c[128dk+p]  (fp8, +-1)
    cbig_d = nc.dram_tensor("cbig", [128, DT * 256], fp8, kind="ExternalInput").ap()
    bvs_d = nc.dram_tensor("bvs", [128, D], bf16, kind="ExternalInput").ap()
    # maskp[p, 256*wq+ct]: keep for boundary s-offset (128*wq+p) vs q col ct
    mask_d = nc.dram_tensor("maskp", [128, 4 * 256], fp16, kind="ExternalInput").ap()
    out_d = nc.dram_tensor("out", [NQ, D], fp16, kind="ExternalOutput").ap()

    with tile.TileContext(nc) as tc:
        with (
            tc.tile_pool(name="const", bufs=1) as constp,
            tc.tile_pool(name="load", bufs=2) as loadp,
            tc.tile_pool(name="vload", bufs=2) as vloadp,
            tc.tile_pool(name="kt", bufs=1) as ktp,
            tc.tile_pool(name="qt", bufs=1) as qtp,
            tc.tile_pool(name="vv", bufs=1) as vvp,
            tc.tile_pool(name="at", bufs=1) as atp,
            tc.tile_pool(name="ps", bufs=3, space="PSUM") as psp,
            tc.tile_pool(name="po", bufs=4, space="PSUM") as pop,
            tc.tile_pool(name="outb", bufs=3) as outp,
        ):
            # ---- constants ----
            # All input DMAs go on the sync queue: transfers serialize on the
            # shared DMA engines in acquisition order, so a single queue in
            # emission order is the only way to control transfer priority
            # (DMAs on other queues race past stalled compute ops).
            cbig_sb = constp.tile([128, DT, 256], fp8, tag="cbig")
            nc.sync.dma_start(cbig_sb[:], cbig_d.rearrange("p (dk j) -> p dk j", dk=DT))
            mask_sb = constp.tile([128, 4 * 256], fp16, tag="maskp")
            bvs_sb = constp.tile([128, D], bf16, tag="bvs")

            def load_consts2():
                nc.sync.dma_start(mask_sb[:], mask_d)
                nc.sync.dma_start(bvs_sb[:], bvs_d)

            # ---- persistent arrays ----
            # skt8[q]: [128 d-part, dk, 512] fp8 = sign(x)^T for s block q;
            # 3D so DoubleRow matmuls can take [:, 2e:2e+2, cols] slices.
            skt8 = [ktp.tile([128, DT, 512], fp8, tag=f"skt{q}", name=f"skt{q}")
                    for q in range(4)]
            # scq8: [128 d-part, dk, 1024] fp8; [:, dk, 256g+ct] = c*S^T at
            # q col (256g+ct)
            scq8 = qtp.tile([128, DT, NQ], fp8, tag="scq")
            # V[i]: [128 s-part, j, 1024 d] fp16 for s-tiles 4i+j
            vt = [vvp.tile([128, 4, D], fp16, tag=f"v{i}", name=f"v{i}")
                  for i in range(4)]
            # attnT[ss]: [128 s-part, 1024 q] fp16
            att = [atp.tile([128, NQ], fp16, tag=f"att{ss}", name=f"att{ss}")
                   for ss in range(ST)]

            xtts = {}
            xnts = {}

            def load_block_dma(q):
                """4 chunked DMAs of s-block q (2 d-tiles each)."""
                xtt = loadp.tile([128, DT, 512], bf16, tag="xtt", name=f"xtt{q}")
                xtts[q] = xtt
                for e in range(4):
                    nc.sync.dma_start(
                        xtt[:, 2 * e:2 * e + 2, :],
                        xtb_d[q][:, 2 * e * 512:(2 * e + 2) * 512])

            def block_compute(q):
                """Per-chunk sign + scq8 for s-block q."""
                xtt = xtts[q]
                for e in range(4):
                    nc.scalar.activation(skt8[q][:, 2 * e:2 * e + 2, :],
                                         xtt[:, 2 * e:2 * e + 2, :], AF.Sign)
                    # q-cols of group g=q are the first 256 cols of this block
                    nc.vector.tensor_mul(
                        scq8[:, 2 * e:2 * e + 2, q * 256:(q + 1) * 256],
                        skt8[q][:, 2 * e:2 * e + 2, 0:256],
                        cbig_sb[:, 2 * e:2 * e + 2, :])

            def load_vblock_dma(i):
                xnt = vloadp.tile([128, 4, D], bf16, tag="xnt", name=f"xnt{i}")
                xnts[i] = xnt
                nc.sync.dma_start(xnt[:], xnb_d[i])

            def vmuls(i):
                for j in range(4):
                    nc.vector.tensor_mul(vt[i][:, j, :], xnts[i][:, j, :],
                                         bvs_sb[:])

            def score_unit(ss, g):
                """scoresT rows s=[128ss,128ss+128) x q cols [256g, 256g+256)."""
                qb = ss // 4
                wq = ss % 4
                co = wq * 128
                ps = psp.tile([128, 256], fp32, tag="ps", name=f"ps{ss}_{g}")
                for e in range(DT // 2):
                    nc.tensor.matmul(
                        ps[:],
                        skt8[qb][:, 2 * e:2 * e + 2, co:co + 128],
                        scq8[:, 2 * e:2 * e + 2, g * 256:(g + 1) * 256],
                        start=(e == 0),
                        stop=(e == DT // 2 - 1),
                        perf_mode=DR,
                    )
                dst = att[ss][:, g * 256:(g + 1) * 256]
                # attn = sigmoid(scores/32 * 4)
                nc.scalar.activation(dst, ps[:], AF.Sigmoid, scale=0.125)
                if g == qb:
                    # boundary chunk: apply causal mask
                    nc.vector.tensor_mul(dst, dst,
                                         mask_sb[:, wq * 256:(wq + 1) * 256])

            def av(ts):
                """output rows t=[128ts,128ts+128): accumulate over s prefix."""
                j = ts // 2
                nss = 4 * (j + 1)
                ob = outp.tile([128, D], fp16, tag="ob", name=f"ob{ts}")
                for dh in range(2):
                    po = pop.tile([128, 512], fp32, tag="po", name=f"po{ts}_{dh}")
                    for ss in range(nss):
                        nc.tensor.matmul(
                            po[:],
                            att[ss][:, ts * 128:(ts + 1) * 128],
                            vt[ss // 4][:, ss % 4, dh * 512:(dh + 1) * 512],
                            start=(ss == 0),
                            stop=(ss == nss - 1),
                        )
                    nc.vector.tensor_copy(ob[:, dh * 512:(dh + 1) * 512], po[:])
                    nc.gpsimd.dma_start(
                        out_d[ts * 128:(ts + 1) * 128,
                              dh * 512:(dh + 1) * 512],
                        ob[:, dh * 512:(dh + 1) * 512])

            # ---- emission order ----
            # Ascending s-blocks. Stage q: load block q (building scq for
            # group g=q), then score all s-tiles ss <= 4q+3 against q-group
            # g=q (their skt blocks are already resident), then the two AV
            # row-tiles of group q, which depend only on those scores. This
            # overlaps AV matmuls with later block DMA instead of
            # serializing all AV at the tail. Block 0 is split into 4
            # DMA/sign chunks so the first scores start early.
            # Sync-queue transfer order: cbig, block0 chunks, xnb0,
            # masks/bvs, then per stage q: xtb(q+1) chunks, xnb(q+1) — so
            # stage-q data always lands before the PE drains stage q-1's
            # work. Each engine executes strictly in order, so per-engine
            # emission must match data-arrival order: the next block's
            # sign/scq (ACT/DVE) are emitted mid-stage, after the current
            # stage's first sigmoids/V-muls.
            load_block_dma(0)
            block_compute(0)
            load_vblock_dma(0)
            load_consts2()
            for q in range(4):
                for ss in range(4 * q + 4):
                    score_unit(ss, q)
                    if ss == 3:
                        vmuls(q)
                        if q < 3:
                            load_block_dma(q + 1)
                            block_compute(q + 1)
                            load_vblock_dma(q + 1)
                av(2 * q)
                av(2 * q + 1)

    nc.compile()
    return nc


def host_inputs(x, bv_q, bv_k, bv_v):
    """Build per-core input maps (all host work is a cast/copy or O(small))."""
    import ml_dtypes
    bfloat16 = ml_dtypes.bfloat16
    f8 = ml_dtypes.float8_e4m3fn

    x = np.asarray(x, dtype=np.float32)
    sq = np.sign(np.asarray(bv_q, dtype=np.float32))
    sk = np.sign(np.asarray(bv_k, dtype=np.float32))
    sv = np.sign(np.asarray(bv_v, dtype=np.float32))
    c = (sq * sk).astype(np.float32)                     # [D]
    # cbig[p, dk*256+j] = c[128dk+p]
    cbig = np.ascontiguousarray(
        np.broadcast_to(c.reshape(DT, 128).T[:, :, None],
                        (128, DT, 256)).reshape(128, DT * 256)).astype(f8)
    bvs = np.ascontiguousarray(
        np.broadcast_to(sv, (128, D))).astype(bfloat16)

    masks = {}
    for parity in (0, 1):
        wo = np.arange(512)[:, None]                     # boundary s offset
        ct = np.arange(256)[None, :]                     # q col offset in group
        if parity == 0:
            keep = wo <= ct                              # orig offsets equal
        else:
            so = np.where(wo < 256, wo + 256, wo - 256)  # swapped halves
            keep = so <= ct + 256
        # [wq*128+p, ct] -> [p, wq*256+ct]
        m = keep.astype(np.float16).reshape(4, 128, 256)
        masks[parity] = np.ascontiguousarray(
            m.transpose(1, 0, 2).reshape(128, 4 * 256))

    in_maps = []
    for core in range(NCORES):
        b, parity = core // 2, core % 2
        xb = x[b]
        if parity == 0:
            xkc = xb
        else:
            xkc = xb.reshape(NG, 2, 256, D)[:, ::-1].reshape(T, D)
        # xtb[q, p, dk, s] = xkc[512q+s, 128dk+p]
        xtb = np.ascontiguousarray(
            xkc.reshape(4, 512, DT, 128).transpose(0, 3, 2, 1)
        ).astype(bfloat16).reshape(4, 128, DT * 512)
        # xnb[i, p, j, d] = xkc[512i+128j+p, d]
        xnb = np.ascontiguousarray(
            xkc.reshape(4, 4, 128, D).transpose(0, 2, 1, 3)
        ).astype(bfloat16).reshape(4, 128, 4 * D)
        in_maps.append({
            "xtb": xtb,
            "xnb": xnb,
            "cbig": cbig,
            "bvs": bvs,
            "maskp": masks[parity],
        })
    return in_maps


def assemble_output(results):
    out = np.zeros((B, T, D), np.float32)
    for core in range(NCORES):
        b, parity = core // 2, core % 2
        o = np.asarray(results[core]["out"], dtype=np.float32).reshape(NG, 256, D)
        for j in range(NG):
            r0 = 512 * j + 256 * parity
            out[b, r0:r0 + 256] = o[j]
    return out


def kernel(x, bv_q, bv_k, bv_v):
    from concourse.bass_utils import run_bass_kernel_spmd

    if "nc" not in _CACHE:
        _CACHE["nc"] = build_nc()
    nc = _CACHE["nc"]

    in_maps = host_inputs(x, bv_q, bv_k, bv_v)
    res = run_bass_kernel_spmd(nc, in_maps, list(range(NCORES)))
    _CACHE["last_result"] = res
    return assemble_output(res.results)
